# revision 32
# baseline (speedup 1.0000x reference)
"""Trainium2 Bass kernel for nn_DiscriminationLoss (segment_reduce).

Math: the loss depends on the input only through, per batch image b:
  - per-kernel per-channel segment sums   s[b, k, c] = sum_p pred[b,c,p] * [lab[b,p]==k]
  - per-kernel pixel counts               n[b, k]    = sum_p [lab[b,p]==k]
  - num_kernel[b] = max label             (= max k with n[b,k] > 0)
followed by a tiny closed-form scalar reduction (see _finalize).

Device strategy (8 cores, data-parallel over batch, one image per core):
  Pixels laid out as [R=128 partitions, Q=3200 columns], grouped into 200
  superchunks of J=16 columns.  For each superchunk, one matmul accumulates
  the 128x144 outer-product block
      acc[(c,j), (k,j')] += sum_r pred[r, sc, c, j] * onehot[r, sc, k, j']
  (pred host-permuted so each superchunk's weights are one contiguous
  [128,128] slice; onehot[r,sc,k,j] = [lab==k] is built on the DVE with
  is_equal, interleaved so each superchunk's rhs is contiguous [128,144]).
  All 200 matmuls accumulate into a single PSUM tile; the host extracts the
  block diagonal j==j' and sums over j, giving s exactly.  Counts come for
  free from the is_equal instructions' accum_out ([P,1] per-partition sums),
  summed on the host (exact integers in fp32).
"""

import numpy as np
from contextlib import ExitStack

import concourse.bass as bass  # noqa: F401
import concourse.tile as tile
from concourse import bacc, mybir
from concourse.bass_utils import run_bass_kernel_spmd

# Problem constants (hardcoded; harness contract).
B, C, H, W = 8, 8, 640, 640
P_PIX = H * W          # 409600
R = 128                # SBUF partitions
Q = P_PIX // R         # 3200
KP1 = 9                # labels 0..8
SIGMA = 3.0
J = 16                 # columns per matmul -> M = 8*16 = 128, N = 9*16 = 144
M = C * J              # 128
N = KP1 * J            # 144
NSC = Q // J           # 200 superchunks total
# Chunk sizes in superchunks: small first (fast pipeline fill) and small
# last (fast drain); sums to NSC.
CHUNK_SIZES = [4, 8, 24, 24, 24, 24, 24, 24, 16, 14, 8, 4, 2]
assert sum(CHUNK_SIZES) == NSC

_cached_nc = None


def _build_program():
    nc = bacc.Bacc(
        "TRN2",
        target_bir_lowering=False,
        debug=False,
        enable_asserts=False,
        num_devices=B,
    )
    pred_d = nc.dram_tensor(
        "pred", [R, NSC, C, J], mybir.dt.float32, kind="ExternalInput"
    )
    lab_d = nc.dram_tensor("lab", [R, Q], mybir.dt.float32, kind="ExternalInput")
    out_d = nc.dram_tensor("out", [M, N], mybir.dt.float32, kind="ExternalOutput")
    nchunk = len(CHUNK_SIZES)
    cnt_d = nc.dram_tensor(
        "cnt", [R, nchunk * KP1], mybir.dt.float32, kind="ExternalOutput"
    )

    with tile.TileContext(nc) as tc, ExitStack() as ctx:
        pred_pool = ctx.enter_context(tc.tile_pool(name="pred", bufs=5))
        predb_pool = ctx.enter_context(tc.tile_pool(name="predb", bufs=5))
        labb_pool = ctx.enter_context(tc.tile_pool(name="labb", bufs=5))
        oh_pool = ctx.enter_context(tc.tile_pool(name="oh", bufs=5))
        psum_pool = ctx.enter_context(tc.tile_pool(name="psum", bufs=1, space="PSUM"))
        singles = ctx.enter_context(tc.tile_pool(name="singles", bufs=1))

        acc = psum_pool.tile([M, N], mybir.dt.float32)
        cnt = singles.tile([R, nchunk * KP1], mybir.dt.float32)

        pred_ap = pred_d.ap()
        lab_ap = lab_d.ap()

        # All labels resident up front: the one-hot pipeline (ACT cast + DVE
        # is_equal) then runs ahead of the pred stream, so the drain after the
        # last pred DMA is just cast -> matmuls -> copy-out.
        lab_all = singles.tile([R, Q], mybir.dt.float32)
        nc.scalar.dma_start(out=lab_all[:, :], in_=lab_ap[:, :])

        sc0 = 0
        for ci, scc in enumerate(CHUNK_SIZES):
            q0 = sc0 * J

            lt = lab_all[:, q0 : q0 + scc * J].rearrange("r (s j) -> r s j", j=J)

            pt = pred_pool.tile([R, scc, C, J], mybir.dt.float32, tag="pt")
            nc.sync.dma_start(out=pt[:, :, :, :], in_=pred_ap[:, sc0 : sc0 + scc, :, :])

            # bf16 copies (pred on DVE, labels on ScalarE): 4x faster PE
            # streaming + FWL weight loads, and 4x-mode is_equal below.
            ptb = predb_pool.tile([R, scc, C, J], mybir.dt.bfloat16, tag="ptb")
            nc.vector.tensor_copy(out=ptb[:, :, :, :], in_=pt[:, :, :, :])

            ltb = labb_pool.tile([R, scc, J], mybir.dt.bfloat16, tag="ltb")
            nc.scalar.copy(out=ltb[:, :, :], in_=lt[:, :, :])

            oh = oh_pool.tile([R, scc, KP1, J], mybir.dt.bfloat16, tag="oh")
            for k in range(KP1):
                nc.vector.tensor_scalar(
                    out=oh[:, :, k, :],
                    in0=ltb[:, :, :],
                    scalar1=float(k),
                    scalar2=None,
                    op0=mybir.AluOpType.is_equal,
                    op1=mybir.AluOpType.add,
                    accum_out=cnt[:, ci * KP1 + k : ci * KP1 + k + 1],
                )

            for s in range(scc):
                sc = sc0 + s
                nc.tensor.matmul(
                    acc[:, :],
                    lhsT=ptb[:, s, :, :],
                    rhs=oh[:, s, :, :],
                    start=(sc == 0),
                    stop=(sc == NSC - 1),
                )
            sc0 += scc

        # cnt depends only on the is_equal ops (all done well before the pred
        # stream ends) -> ship it early on the ACT ring, off the critical tail.
        nc.scalar.dma_start(out=cnt_d.ap()[:, :], in_=cnt[:])
        ot = singles.tile([M, N], mybir.dt.float32)
        nc.vector.tensor_copy(out=ot[:], in_=acc[:, :])
        nc.sync.dma_start(out=out_d.ap()[:, :], in_=ot[:])

    nc.compile()
    return nc


def _get_program():
    global _cached_nc
    if _cached_nc is None:
        _cached_nc = _build_program()
    return _cached_nc


def _make_in_maps(pred_similarities, kernel_mask_ndi_labels):
    pred = np.asarray(pred_similarities, dtype=np.float32).reshape(B, C, R, NSC, J)
    # [b, c, r, sc, j] -> [b, r, sc, c, j] so each superchunk's weights are a
    # contiguous [128, 128] SBUF slice.
    predperm = np.ascontiguousarray(pred.transpose(0, 2, 3, 1, 4))
    lab = np.asarray(kernel_mask_ndi_labels).reshape(B, R, Q).astype(np.float32)
    return [{"pred": predperm[b], "lab": lab[b]} for b in range(B)]


def _finalize(results):
    """Combine the 8 per-core Gram blocks + counts into the scalar loss."""
    f_sigma = float(np.log(SIGMA**2 + 1.0))
    total = 0.0
    for b in range(B):
        O = np.asarray(results[b]["out"], dtype=np.float64).reshape(C, J, KP1, J)
        S = np.einsum("cjkj->ck", O)  # diagonal j==j', summed over j
        s = S.T                       # [k, c]
        cnt = np.asarray(results[b]["cnt"], dtype=np.float64)
        n = cnt.reshape(R, len(CHUNK_SIZES), KP1).sum(axis=(0, 1))  # [k] exact counts
        present = np.nonzero(n > 0.5)[0]
        num_kernel = int(present.max()) if present.size else 0
        m = float(num_kernel)
        snorm = np.sqrt((s * s).sum(axis=1))       # [k]
        f = np.log(np.maximum(SIGMA - snorm, 0.0) ** 2 + 1.0)
        valid = np.arange(KP1) >= 1
        valid &= np.arange(KP1) <= num_kernel
        per_kernel = float((n * (f - f_sigma))[valid].sum())
        num_pairs = m * (m - 1.0) * 0.5
        total += (m - 1.0) * per_kernel + num_pairs * (B * P_PIX) * f_sigma
    return np.asarray(total, dtype=np.float32)


def kernel(pred_similarities, kernel_mask_ndi_labels):
    nc = _get_program()
    in_maps = _make_in_maps(pred_similarities, kernel_mask_ndi_labels)
    # The axon-tunneled NeuronCores occasionally report a transient
    # NRT_EXEC_UNIT_UNRECOVERABLE wedge from a previously aborted process; a
    # plain retry has always recovered it.
    last_err = None
    for attempt in range(3):
        try:
            res = run_bass_kernel_spmd(nc, in_maps, core_ids=list(range(B)))
            return _finalize(res.results)
        except Exception as e:  # noqa: BLE001 - retry transient device wedges
            last_err = e
            import time

            time.sleep(10 * (attempt + 1))
    raise last_err


def modeled_exec_time_ns():
    """Cost-model (TimelineSim) estimate of per-core HW exec time in ns.

    The axon client in this container has no NTFF profiling hook, so real
    per-kernel HW timing is unavailable; this is the calibrated cost-model
    timeline for the compiled program.
    """
    from concourse.timeline_sim import TimelineSim

    return TimelineSim(_get_program(), trace=False).simulate()
